# revision 1
# baseline (speedup 1.0000x reference)
"""Trainium2 Bass kernel for nn_Cst2Val_Layer (GNN message passing).

Strategy (8 NeuronCores):
  - Shard constraints (cst) and values (val) row-wise: core c owns cst rows
    [c*NCs, (c+1)*NCs) and val rows [c*NVs, (c+1)*NVs).
  - Edges are sharded by SOURCE cst, so each core computes its cst_send MLP
    shard m = LN(ReLU(r_cst@W1+b1)@W2) locally (stored to HBM as [4*NCs, H]
    rows), gathers its edges' message rows with dma_gather, and scatter-adds
    them into a full-size partial r_val accumulator with dma_scatter_add
    (SDMA CCE add).  Scatter calls are built as conflict-free rounds (each
    destination row at most once per call).
  - The partial r_val is reduce-scattered across the 8 cores in J chunks
    (laid out so each RS chunk hands every core a contiguous slice of its
    own val shard); each chunk's RS is issued after the next chunk's first
    few gather/scatter windows so the Pool queue stays fed, and the val_rec
    MLP (phase 3) for that chunk is emitted right after its RS so it
    overlaps the remaining chunks' scatter work.

DMA dispatch is minimized (each HWDGE DMACopy costs ~600ns of global
dispatch serialization): consts are packed into one tensor, the gather/
scatter index streams are SBUF-resident (2 loads), and phase 1/3 move
512-row blocks per DMA using multi-level access patterns.  LayerNorm
stats are batched 4 tiles at a time into [128,4] vectors (one Rsqrt, no
reciprocal) to keep the DVE/Act sequencers off the critical path.
"""

from dataclasses import dataclass

import numpy as np

H = 128
EPS = 1e-5


@dataclass(frozen=True)
class Cfg:
    cores: int = 8
    nc_tot: int = 100000
    nv_tot: int = 100000
    J: int = 4           # number of reduce-scatter chunks
    sub: int = 1024      # max slots per gather/scatter call (>=1280 wedges)
    scratch: int = 65536 # dynamic_dma_scratch_size (ring = scratch//16)
    rs_overlap: int = 6  # windows of chunk j+1 issued before RS(j)
    acc_bf16: bool = True  # bf16 messages/partials (halves zero/store traffic)
    swq: int = 2         # SWDGE queues: gathers on q0, scatters on q(swq-1)
    reps: int = 1        # benchmark only: run the whole kernel body N times
    pad_cols: int = 0    # benchmark only: salt the consts-tensor shape so
                         # each variant gets a distinct neuron-cache entry

    @property
    def NCs(self):
        return self.nc_tot // self.cores      # real cst per core (12500)

    @property
    def NCp(self):
        return 12800                           # padded cst per core

    @property
    def T1(self):
        return self.NCp // 128                 # 100 tiles

    @property
    def M_ROWS(self):
        return 4 * self.NCp                    # 51200

    @property
    def BANK_ROWS(self):
        return self.M_ROWS // 2                # 25600 (int16-safe)

    @property
    def NVs(self):
        return self.nv_tot // self.cores       # real val per core (12500)

    @property
    def NVp(self):
        return 12800                           # padded val per core

    @property
    def S(self):
        return self.NVp // self.J              # 3200 rows per RS chunk

    @property
    def CH_ROWS(self):
        return self.cores * self.S             # 25600

    @property
    def CH_ALLOC(self):
        return -(-(self.CH_ROWS + 1) // 128) * 128  # +dummy row, 25728

    @property
    def T2(self):
        return self.NVp // 128                 # 100


def _round_up(x, m):
    return -(-x // m) * m


def _occurrence_rank(keys):
    """For a SORTED int array, return the occurrence index of each element
    within its run of equal values."""
    n = keys.shape[0]
    if n == 0:
        return np.zeros(0, np.int64)
    new_run = np.empty(n, bool)
    new_run[0] = True
    np.not_equal(keys[1:], keys[:-1], out=new_run[1:])
    run_id = np.cumsum(new_run) - 1
    run_start = np.flatnonzero(new_run)
    return np.arange(n, dtype=np.int64) - run_start[run_id]


def _build_plan(cfg, src, dst, slot):
    """Host-side edge preprocessing.

    Returns (calls, tot_slots, gstream, sstream) where
      calls: per chunk j, list of scatter-call windows
             (slot0, length, [(gslot0, glen, bank), ...])
      gstream/sstream: [cores, tot_slots] int16 gather/scatter index streams.
    """
    C, J, S = cfg.cores, cfg.J, cfg.S
    E = src.shape[0]

    core = src // cfg.NCs
    m_row = 4 * (src % cfg.NCs) + slot          # [0, 4*NCs)
    bank = m_row // cfg.BANK_ROWS               # 0/1
    row_in_bank = m_row % cfg.BANK_ROWS

    dr = dst // cfg.NVs
    off = dst % cfg.NVs
    j = off // S
    k = off % S
    scat_local = dr * S + k                     # [0, CH_ROWS)

    # conflict-free round number: occurrence rank of (core, j, scat_local)
    comb = (core * J + j) * cfg.CH_ROWS + scat_local
    order = np.argsort(comb, kind="stable")
    rank_sorted = _occurrence_rank(comb[order])
    rank = np.empty(E, np.int64)
    rank[order] = rank_sorted

    # segment = (j, r, s); per-core counts -> capped segment sizes
    n_r = int(rank.max()) + 1 if E else 1
    seg_of_edge = (j * n_r + rank) * 2 + bank
    n_seg = J * n_r * 2
    counts = np.zeros((C, n_seg), np.int64)
    np.add.at(counts, (core, seg_of_edge), 1)
    cap = counts.max(axis=0)                    # [n_seg]
    cap = np.where(cap > 0, _round_up(np.maximum(cap, 1), 128), 0)

    # lay out segments: per chunk j, rounds in order, banks in order
    seg_base = np.zeros(n_seg, np.int64)
    calls = [[] for _ in range(J)]
    pos = 0
    for jj in range(J):
        for r in range(n_r):
            s_ids = [(jj * n_r + r) * 2 + 0, (jj * n_r + r) * 2 + 1]
            lens = [int(cap[s_ids[0]]), int(cap[s_ids[1]])]
            if lens[0] + lens[1] == 0:
                continue
            round_base = pos
            seg_base[s_ids[0]] = pos
            seg_base[s_ids[1]] = pos + lens[0]
            L = lens[0] + lens[1]
            pos += L
            # windows of <= sub slots; all boundaries are multiples of 128
            w0 = 0
            while w0 < L:
                wl = min(cfg.sub, L - w0)
                gathers = []
                for b in (0, 1):
                    gs = [0, lens[0]][b]
                    ge = gs + lens[b]
                    a = max(w0, gs)
                    e_ = min(w0 + wl, ge)
                    if e_ > a:
                        gathers.append((round_base + a, e_ - a, b))
                calls[jj].append((r, round_base + w0, wl, gathers))
                w0 += wl
    tot = pos

    # fill per-core streams
    # position of each edge: seg_base[seg] + occurrence rank within (core, seg)
    comb2 = core * n_seg + seg_of_edge
    order2 = np.argsort(comb2, kind="stable")
    within = np.empty(E, np.int64)
    within[order2] = _occurrence_rank(comb2[order2])
    epos = seg_base[seg_of_edge] + within

    gstream = np.zeros((C, tot), np.int16)                  # pad: bank row 0
    sstream = np.full((C, tot), cfg.CH_ROWS, np.int16)      # pad: dummy row
    gstream[core, epos] = row_in_bank.astype(np.int16)
    sstream[core, epos] = scat_local.astype(np.int16)
    return calls, tot, gstream, sstream


def _idx_layout(stream):
    """[tot] int16 -> [128, tot//16] wrapped+replicated layout."""
    tot = stream.shape[0]
    assert tot % 16 == 0
    base = stream.reshape(tot // 16, 16).T  # [16, tot/16]
    return np.ascontiguousarray(np.tile(base, (8, 1)))


def _build_module(cfg, tot_slots, calls, apply_gb1=False, apply_gb2=False):
    import concourse.bacc as bacc
    import concourse.mybir as mybir
    from concourse import tile

    f32 = mybir.dt.float32
    i16 = mybir.dt.int16
    acc_dt = mybir.dt.bfloat16 if cfg.acc_bf16 else mybir.dt.float32
    AF = mybir.ActivationFunctionType
    ALU = mybir.AluOpType

    nc = bacc.Bacc(
        "TRN2",
        target_bir_lowering=False,
        debug=False,
        num_devices=cfg.cores,
        dynamic_dma_scratch_size=cfg.scratch,
        num_swdge_queues=cfg.swq,
    )

    # packed consts layout (columns in a [128, CW] f32 tensor)
    co = {}
    off = 0
    for name, w in [
        ("W1", H), ("b1", 1), ("W2", 4 * H), ("W3", H), ("b3", 1),
        ("W4", H), ("id", H),
    ]:
        co[name] = off
        off += w
    off += cfg.pad_cols
    if apply_gb1:
        co["g1"] = off
        off += 4 * H
        co["bt1"] = off
        off += 4 * H
    if apply_gb2:
        co["g2"] = off
        off += H
        co["bt2"] = off
        off += H
    CW = off

    # ---- I/O ----
    rcT_d = nc.dram_tensor("rcT", [H, cfg.NCp], f32, kind="ExternalInput")
    xv_d = nc.dram_tensor("xv", [cfg.NVp, H], f32, kind="ExternalInput")
    gidx_d = nc.dram_tensor("gidx", [128, tot_slots // 16], i16, kind="ExternalInput")
    sidx_d = nc.dram_tensor("sidx", [128, tot_slots // 16], i16, kind="ExternalInput")
    cst_d = nc.dram_tensor("consts", [128, CW], f32, kind="ExternalInput")
    out_d = nc.dram_tensor("out", [cfg.NVp, H], f32, kind="ExternalOutput")

    # ---- internal DRAM ----
    m_dram = nc.dram_tensor("m_scratch", [cfg.M_ROWS, H], acc_dt)
    partial = [
        nc.dram_tensor(f"partial{j}", [cfg.CH_ALLOC, H], acc_dt)
        for j in range(cfg.J)
    ]
    rs_t = [nc.dram_tensor(f"rs{j}", [cfg.S, H], acc_dt) for j in range(cfg.J)]

    rg = [list(range(cfg.cores))]
    NB1 = cfg.T1 // 4   # 512-cst blocks in phase 1

    with tile.TileContext(nc) as tc:
        with tc.tile_pool(name="consts", bufs=1) as cp:
            cst_s = cp.tile([128, CW], f32)
            nc.sync.dma_start(cst_s[:], cst_d[:])
            W1_s = cst_s[:, co["W1"] : co["W1"] + H]
            b1_s = cst_s[:, co["b1"] : co["b1"] + 1]
            W2_s = cst_s[:, co["W2"] : co["W2"] + 4 * H]
            W3_s = cst_s[:, co["W3"] : co["W3"] + H]
            b3_s = cst_s[:, co["b3"] : co["b3"] + 1]
            W4_s = cst_s[:, co["W4"] : co["W4"] + H]
            id_s = cst_s[:, co["id"] : co["id"] + H]

            gidx_s = cp.tile([128, tot_slots // 16], i16)
            nc.sync.dma_start(gidx_s[:], gidx_d[:])
            sidx_s = cp.tile([128, tot_slots // 16], i16)
            nc.sync.dma_start(sidx_s[:], sidx_d[:])

            eps_s = cp.tile([128, 1], f32)
            nc.vector.memset(eps_s[:], EPS)

            def ln_chain(stp, mv_tile, n, tag):
                """mv [128,*,2] (mean,var) -> (rsig, nmr) [128,n] with
                nmr = -mean*rsig, so normalized = rsig*x + nmr (one Act
                activation with scale/bias per tile)."""
                sd = stp.tile([128, n], f32, tag=f"{tag}_sd")
                nc.scalar.activation(
                    sd[:], mv_tile[:, :n, 1], AF.Sqrt, bias=eps_s[:]
                )
                rsig = stp.tile([128, n], f32, tag=f"{tag}_rsig")
                nc.vector.reciprocal(rsig[:], sd[:])
                nmu = stp.tile([128, n], f32, tag=f"{tag}_nmu")
                nc.vector.tensor_scalar(
                    nmu[:], mv_tile[:, :n, 0], -1.0, None, ALU.mult
                )
                nmr = stp.tile([128, n], f32, tag=f"{tag}_nmr")
                nc.vector.tensor_tensor(nmr[:], nmu[:], rsig[:], ALU.mult)
                return rsig, nmr

            for _rep in range(cfg.reps):
                # ---- zero the partial accumulators (scalar queue,
                # overlaps with phase 1 compute) ----
                with tc.tile_pool(name="zpool", bufs=1) as zp:
                    zero_s = zp.tile([128, 8192], acc_dt)
                    nc.vector.memset(zero_s[:], 0.0)
                    for j in range(cfg.J):
                        base = 0
                        while base < cfg.CH_ALLOC:
                            rows = min(8192, cfg.CH_ALLOC - base)
                            dstv = partial[j][base : base + rows, :].rearrange(
                                "(p a) h -> p (a h)", p=128
                            )
                            nc.scalar.dma_start(dstv, zero_s[:, : rows * H // 128])
                            base += rows

                    # ============ Phase 1: cst_send MLP ============
                    with (
                        tc.tile_pool(name="m1_io", bufs=3) as iop,
                        tc.tile_pool(name="m1_mid", bufs=3) as midp,
                        tc.tile_pool(name="m1_stat", bufs=3) as stp,
                        tc.tile_pool(name="ps1", bufs=2, space="PSUM") as psA,
                        tc.tile_pool(name="ps2", bufs=5, space="PSUM") as psB,
                    ):
                        for blk in range(NB1):
                            xt = iop.tile([128, 512], f32, tag="xt")
                            nc.sync.dma_start(
                                xt[:], rcT_d[:, blk * 512 : (blk + 1) * 512]
                            )
                            ps_h1 = psA.tile([128, 512], f32, tag="ps_h1")
                            nc.tensor.matmul(
                                ps_h1[:], W1_s, xt[:], start=True, stop=True
                            )
                            h1 = midp.tile([128, 512], f32, tag="h1")
                            nc.scalar.activation(h1[:], ps_h1[:], AF.Relu, bias=b1_s)
                            mn4 = midp.tile([128, 2048], acc_dt, tag="mn4")
                            sT = stp.tile([128, 4, 6], f32, tag="sT")
                            mv = stp.tile([128, 4, 2], f32, tag="mv")
                            ps_ms = []
                            for t in range(4):
                                ps_m = psB.tile([128, 512], f32, tag="ps_m")
                                ps_ms.append(ps_m)
                                nc.tensor.matmul(
                                    ps_m[:],
                                    h1[:, t * 128 : (t + 1) * 128],
                                    W2_s,
                                    start=True,
                                    stop=True,
                                )
                                nc.vector.bn_stats(sT[:, t, :], ps_m[:])
                                nc.vector.bn_aggr(mv[:, t, :], sT[:, t, :])
                            rsig, nmr = ln_chain(stp, mv, 4, "p1")
                            for t in range(4):
                                mnv = mn4[:, t * 512 : (t + 1) * 512]
                                nc.scalar.activation(
                                    mnv, ps_ms[t][:], AF.Identity,
                                    bias=nmr[:, t : t + 1],
                                    scale=rsig[:, t : t + 1],
                                )
                                if apply_gb1:
                                    g1_s = cst_s[:, co["g1"] : co["g1"] + 4 * H]
                                    bt1_s = cst_s[:, co["bt1"] : co["bt1"] + 4 * H]
                                    nc.vector.tensor_tensor(mnv, mnv, g1_s, ALU.mult)
                                    nc.vector.tensor_tensor(mnv, mnv, bt1_s, ALU.add)
                            # one DMA stores all 4 tiles (2048 m rows)
                            dstv = m_dram[blk * 2048 : (blk + 1) * 2048, :].rearrange(
                                "(t c s) h -> c t (s h)", t=4, c=128, s=4
                            )
                            nc.sync.dma_start(
                                dstv, mn4[:].rearrange("c (t f) -> c t f", t=4)
                            )

                # ==== Phase 2 (gather/scatter + RS) interleaved with
                # ==== Phase 3 (val_rec MLP per finished chunk) ====
                with (
                    tc.tile_pool(name="slots", bufs=3) as sp,
                    tc.tile_pool(name="m2_io", bufs=3) as iop2,
                    tc.tile_pool(name="m2_mid", bufs=3) as midp2,
                    tc.tile_pool(name="m2_stat", bufs=3) as stp2,
                    tc.tile_pool(name="ps3", bufs=2, space="PSUM") as psC,
                    tc.tile_pool(name="ps4", bufs=2, space="PSUM") as psD,
                    tc.tile_pool(name="ps5", bufs=2, space="PSUM") as psE,
                ):

                    def phase3_chunk(j):
                        """val_rec MLP on this core's chunk-j rows."""
                        o = 0
                        while o < cfg.S:
                            nt = min(4, (cfg.S - o) // 128)
                            F = nt * 128
                            r0 = j * cfg.S + o
                            xt2 = iop2.tile([128, 512], f32, tag="xt2")
                            nc.sync.dma_start(
                                xt2[:, :F].rearrange("p (a h) -> p a h", a=nt),
                                xv_d[r0 : r0 + F, :].rearrange(
                                    "(a p) h -> p a h", p=128
                                ),
                            )
                            rv = iop2.tile([128, 512], acc_dt, tag="rv")
                            nc.sync.dma_start(
                                rv[:, :F].rearrange("p (a h) -> p a h", a=nt),
                                rs_t[j][o : o + F, :].rearrange(
                                    "(a p) h -> p a h", p=128
                                ),
                            )
                            hh = midp2.tile([128, 512], f32, tag="hh")
                            nc.vector.tensor_tensor(
                                hh[:, :F], xt2[:, :F], rv[:, :F], ALU.add
                            )
                            ps_hT = psC.tile([128, 512], f32, tag="ps_hT")
                            for i in range(nt):
                                nc.tensor.transpose(
                                    ps_hT[:, i * 128 : (i + 1) * 128],
                                    hh[:, i * 128 : (i + 1) * 128],
                                    id_s,
                                )
                            hT = midp2.tile([128, 512], f32, tag="hT")
                            nc.scalar.copy(hT[:, :F], ps_hT[:, :F])
                            ps_h2 = psD.tile([128, 512], f32, tag="ps_h2")
                            nc.tensor.matmul(
                                ps_h2[:, :F], W3_s, hT[:, :F], start=True, stop=True
                            )
                            h2 = midp2.tile([128, 512], f32, tag="h2")
                            nc.scalar.activation(
                                h2[:, :F], ps_h2[:, :F], AF.Relu, bias=b3_s
                            )
                            ps_o = psE.tile([128, 512], f32, tag="ps_o")
                            sT2 = stp2.tile([128, 4, 6], f32, tag="sT2")
                            mv2 = stp2.tile([128, 4, 2], f32, tag="mv2")
                            for i in range(nt):
                                nc.tensor.matmul(
                                    ps_o[:, i * 128 : (i + 1) * 128],
                                    h2[:, i * 128 : (i + 1) * 128],
                                    W4_s,
                                    start=True,
                                    stop=True,
                                )
                                po = ps_o[:, i * 128 : (i + 1) * 128]
                                nc.vector.bn_stats(sT2[:, i, :], po)
                                nc.vector.bn_aggr(mv2[:, i, :], sT2[:, i, :])
                            rsig2, nmr2 = ln_chain(stp2, mv2, nt, "p3")
                            on = midp2.tile([128, 512], f32, tag="on")
                            for i in range(nt):
                                onv = on[:, i * 128 : (i + 1) * 128]
                                nc.scalar.activation(
                                    onv,
                                    ps_o[:, i * 128 : (i + 1) * 128],
                                    AF.Identity,
                                    bias=nmr2[:, i : i + 1],
                                    scale=rsig2[:, i : i + 1],
                                )
                                if apply_gb2:
                                    g2_s = cst_s[:, co["g2"] : co["g2"] + H]
                                    bt2_s = cst_s[:, co["bt2"] : co["bt2"] + H]
                                    nc.vector.tensor_tensor(onv, onv, g2_s, ALU.mult)
                                    nc.vector.tensor_tensor(onv, onv, bt2_s, ALU.add)
                            nc.vector.tensor_tensor(
                                on[:, :F], on[:, :F], xt2[:, :F], ALU.add
                            )
                            nc.scalar.dma_start(
                                out_d[r0 : r0 + F, :].rearrange(
                                    "(a p) h -> p a h", p=128
                                ),
                                on[:, :F].rearrange("p (a h) -> p a h", a=nt),
                            )
                            o += F

                    def emit_rs(j):
                        nc.gpsimd.collective_compute(
                            "ReduceScatter",
                            mybir.AluOpType.add,
                            replica_groups=rg,
                            ins=[partial[j][0 : cfg.CH_ROWS, :]],
                            outs=[rs_t[j][:, :]],
                        )
                        phase3_chunk(j)

                    def emit_gathers(st, slot0, gathers, widx=0):
                        gq = widx % 2 if cfg.swq >= 4 else 0
                        for gs0, glen, b in gathers:
                            c0 = (gs0 - slot0) // 128
                            nc.gpsimd.dma_gather(
                                st[:, c0 : c0 + glen // 128, :],
                                m_dram[
                                    b * cfg.BANK_ROWS : (b + 1) * cfg.BANK_ROWS, :
                                ],
                                gidx_s[:, gs0 // 16 : (gs0 + glen) // 16],
                                glen,
                                glen,
                                H,
                                queue_num=gq,
                            )

                    def emit_scatter(j, st, slot0, wl, widx=0):
                        sq_ = (2 + widx % 2) if cfg.swq >= 4 else cfg.swq - 1
                        nc.gpsimd.dma_scatter_add(
                            partial[j][:, :],
                            st[:, : wl // 128, :],
                            sidx_s[:, slot0 // 16 : (slot0 + wl) // 16],
                            wl,
                            wl,
                            H,
                            queue_num=sq_,
                        )

                    # Emission sequence: weighted round-robin.  Scatter calls
                    # to the same partial[] are WAW-serialized by Tile (CCE
                    # adds to the same row must not race), so consecutive
                    # windows of one chunk stall on each other's DMA
                    # completion.  Alternating the oldest active chunk with a
                    # rotation over the others keeps every chunk's scatter
                    # chain spaced out while still draining chunks in order
                    # (staggered completions keep the reduce-scatters off the
                    # critical path).
                    seq = []
                    pos = [0] * cfg.J
                    active = [j for j in range(cfg.J) if calls[j]]
                    k = 0
                    while active:
                        if len(active) == 1 or k % 2 == 0:
                            c = active[0]
                        else:
                            others = active[1:]
                            c = others[(k // 2) % len(others)]
                        seq.append((c, calls[c][pos[c]]))
                        pos[c] += 1
                        if pos[c] == len(calls[c]):
                            active.remove(c)
                        k += 1

                    # software-pipelined one deep: the gather of window w+1
                    # is issued before the scatter of window w so the Pool
                    # engine never stalls on the gather DMA completing.
                    left = [len(calls[j]) for j in range(cfg.J)]
                    rs_delay = {}  # chunk -> windows until RS emit
                    prev = None

                    def tick_rs():
                        for jj in sorted(rs_delay):
                            rs_delay[jj] -= 1
                            if rs_delay[jj] <= 0:
                                del rs_delay[jj]
                                emit_rs(jj)

                    for wi, (j, (r, slot0, wl, gathers)) in enumerate(seq):
                        st = sp.tile(
                            [128, cfg.sub // 128, 128], acc_dt, tag="slots"
                        )
                        emit_gathers(st, slot0, gathers, wi)
                        if prev is not None:
                            pj = prev[0]
                            emit_scatter(*prev)
                            left[pj] -= 1
                            if left[pj] == 0:
                                rs_delay[pj] = 3
                        prev = (j, st, slot0, wl, wi)
                        tick_rs()
                    if prev is not None:
                        pj = prev[0]
                        emit_scatter(*prev)
                        for jj in sorted(rs_delay):
                            emit_rs(jj)
                        emit_rs(pj)

    nc.compile()
    return nc


def co_pad_base(cfg):
    # width of the always-present consts block (W1,b1,W2,W3,b3,W4,id)
    return H + 1 + 4 * H + H + 1 + H + H


def _prep_inputs(cfg, inputs):
    """Host-side sharding; returns (in_maps, tot_slots, calls, gb1, gb2)."""
    x_val = np.ascontiguousarray(np.asarray(inputs["x_val"], np.float32))
    r_cst = np.ascontiguousarray(np.asarray(inputs["r_cst"], np.float32))
    edges = np.asarray(inputs["cst_edges"]).astype(np.int64)
    le = np.asarray(inputs["LE"]).astype(np.int64)
    pe = np.asarray(inputs["PE"]).astype(np.int64)
    slot = 2 * le + pe

    calls, tot, gstream, sstream = _build_plan(cfg, edges[0], edges[1], slot)
    tot = max(tot, 128)

    W1 = np.asarray(inputs["W1"], np.float32)
    b1 = np.asarray(inputs["b1"], np.float32).reshape(H, 1)
    W2 = np.asarray(inputs["W2"], np.float32)
    W3 = np.asarray(inputs["W3"], np.float32)
    b3 = np.asarray(inputs["b3"], np.float32).reshape(H, 1)
    W4 = np.asarray(inputs["W4"], np.float32)
    g1 = np.asarray(inputs["g1"], np.float32)
    bt1 = np.asarray(inputs["bt1"], np.float32)
    g2 = np.asarray(inputs["g2"], np.float32)
    bt2 = np.asarray(inputs["bt2"], np.float32)
    apply_gb1 = not (np.all(g1 == 1.0) and np.all(bt1 == 0.0))
    apply_gb2 = not (np.all(g2 == 1.0) and np.all(bt2 == 0.0))

    parts = [W1, b1, W2, W3, b3, W4, np.eye(128, dtype=np.float32)]
    if apply_gb1:
        parts += [
            np.broadcast_to(g1, (128, 4 * H)),
            np.broadcast_to(bt1, (128, 4 * H)),
        ]
    if apply_gb2:
        parts += [
            np.broadcast_to(g2, (128, H)),
            np.broadcast_to(bt2, (128, H)),
        ]
    consts = np.concatenate(parts, axis=1, dtype=np.float32)
    if cfg.pad_cols:
        pad = np.zeros((128, cfg.pad_cols), np.float32)
        consts = np.concatenate(
            [consts[:, : co_pad_base(cfg)], pad, consts[:, co_pad_base(cfg) :]],
            axis=1,
        )
    consts = np.ascontiguousarray(consts)

    in_maps = []
    for c in range(cfg.cores):
        rc = r_cst[c * cfg.NCs : (c + 1) * cfg.NCs]
        rcT = np.zeros((H, cfg.NCp), np.float32)
        rcT[:, : cfg.NCs] = rc.T
        xv = np.zeros((cfg.NVp, H), np.float32)
        xv[: cfg.NVs] = x_val[c * cfg.NVs : (c + 1) * cfg.NVs]
        gs = np.zeros(tot, np.int16)
        ss = np.full(tot, cfg.CH_ROWS, np.int16)
        gs[: gstream.shape[1]] = gstream[c]
        ss[: sstream.shape[1]] = sstream[c]
        in_maps.append(
            {
                "rcT": np.ascontiguousarray(rcT),
                "xv": xv,
                "gidx": _idx_layout(gs),
                "sidx": _idx_layout(ss),
                "consts": consts,
            }
        )
    return in_maps, tot, calls, apply_gb1, apply_gb2


def run(inputs, cfg=None, trace=False):
    """Build, run on hardware, return (output, BassKernelResults)."""
    from concourse.bass_utils import run_bass_kernel_spmd

    cfg = cfg or Cfg()
    in_maps, tot, calls, gb1, gb2 = _prep_inputs(cfg, inputs)
    nc = _build_module(cfg, tot, calls, gb1, gb2)
    res = run_bass_kernel_spmd(
        nc, in_maps, core_ids=list(range(cfg.cores)), trace=trace
    )
    out = np.concatenate(
        [res.results[c]["out"][: cfg.NVs] for c in range(cfg.cores)], axis=0
    )
    return out, res


def kernel(**inputs) -> np.ndarray:
    out, _ = run(inputs)
    return out



# revision 2
# speedup vs baseline: 1.0056x; 1.0056x over previous
"""Trainium2 Bass kernel for nn_Cst2Val_Layer (GNN message passing), v3.

Strategy (8 NeuronCores):
  - Shard constraints (cst) and values (val) row-wise as in the baseline;
    core c computes its cst_send MLP shard m (bf16, [51200, H] in DRAM,
    2 int16-safe banks) and gathers its edges' message rows with SWDGE
    dma_gather -- but the gather stream is sorted by DESTINATION
    (chunk j, bank, scat_local), so the gathered tiles arrive grouped by
    128-row destination windows.
  - The segment-sum is done on the Tensor engine: for each 128-edge tile,
    a one-hot matrix A[e, d] = (dst_local(e) == d) is built on DVE with a
    single is_equal against an iota row (codes shipped from the host as a
    bf16 [128, ncols] tensor), and psum_w += A.T @ msg accumulates into a
    per-window PSUM accumulator.  Finished windows are added into an SBUF
    chunk buffer (DVE), which is densely DMA'd to the partial[j] DRAM
    tensor.  This removes the baseline's dma_scatter_add entirely (no
    descriptor generation, no conflict rounds, no zeroing, no RMW DMA).
  - SPMD: all 8 cores run one program, so each (chunk, bank, window)
    segment is padded to the max count over cores (pad slots gather row 0
    and carry code 255, which the one-hot maps to zero columns).
  - partial[j] is reduce-scattered as in the baseline; the val_rec MLP
    (phase 3) runs per chunk right after its RS.
"""

from dataclasses import dataclass

import numpy as np

H = 128
EPS = 1e-5


@dataclass(frozen=True)
class Cfg:
    cores: int = 8
    nc_tot: int = 100000
    nv_tot: int = 100000
    J: int = 4           # number of reduce-scatter chunks
    sub: int = 1024      # max slots per gather call
    scratch: int = 65536 # dynamic_dma_scratch_size
    swq: int = 2         # SWDGE queues for gathers
    psum_bufs: int = 4   # window accumulators in PSUM

    @property
    def NCs(self):
        return self.nc_tot // self.cores      # 12500

    @property
    def NCp(self):
        return 12800

    @property
    def T1(self):
        return self.NCp // 128                 # 100

    @property
    def M_ROWS(self):
        return 4 * self.NCp                    # 51200

    @property
    def BANK_ROWS(self):
        return self.M_ROWS // 2                # 25600

    @property
    def NVs(self):
        return self.nv_tot // self.cores       # 12500

    @property
    def NVp(self):
        return 12800

    @property
    def S(self):
        return self.NVp // self.J              # 3200

    @property
    def CH_ROWS(self):
        return self.cores * self.S             # 25600

    @property
    def NWIN(self):
        return self.CH_ROWS // 128             # 200

    @property
    def T2(self):
        return self.NVp // 128                 # 100


def _round_up(x, m):
    return -(-x // m) * m


def _occurrence_rank(keys):
    n = keys.shape[0]
    if n == 0:
        return np.zeros(0, np.int64)
    new_run = np.empty(n, bool)
    new_run[0] = True
    np.not_equal(keys[1:], keys[:-1], out=new_run[1:])
    run_id = np.cumsum(new_run) - 1
    run_start = np.flatnonzero(new_run)
    return np.arange(n, dtype=np.int64) - run_start[run_id]


def _build_plan(cfg, src, dst, slot):
    """Host-side plan: dst-sorted gather stream + one-hot codes + schedule.

    Returns (sched, tots, gstream, codes, ncols) where
      sched: per chunk j, list over banks of
             (bank, ntiles, [per tile: [(win, col)...]], {win: (first, last)})
             -- first/last are (tile_idx, target_idx) pairs marking psum
             start/stop boundaries per window within this (j, bank).
      tots:  per (j, b) padded slot counts (each a multiple of 128).
      gstream: [cores, TOT] int16 gather row-in-bank stream.
      codes:  [cores, 128, ncols] float32 one-hot codes (255 = no match).
    """
    C, J = cfg.cores, cfg.J
    E = src.shape[0]
    NW = cfg.NWIN

    core = src // cfg.NCs
    m_row = 4 * (src % cfg.NCs) + slot
    bank = m_row // cfg.BANK_ROWS
    rib = m_row % cfg.BANK_ROWS

    dr = dst // cfg.NVs
    off = dst % cfg.NVs
    j = off // cfg.S
    k = off % cfg.S
    scat = dr * cfg.S + k
    win = scat // 128
    dloc = scat % 128

    # counts per (core, j, bank, win) -> cap = max over cores
    flat = ((core * J + j) * 2 + bank) * NW + win
    counts = np.bincount(flat, minlength=C * J * 2 * NW).reshape(C, J, 2, NW)
    cap = counts.max(axis=0)                   # [J, 2, NW]

    # segment bases within each (j, b) stream; pad stream to x128
    tots = np.zeros((J, 2), np.int64)
    base = np.zeros((J, 2, NW), np.int64)
    for jj in range(J):
        for b in range(2):
            c = np.cumsum(cap[jj, b])
            base[jj, b, 0] = 0
            base[jj, b, 1:] = c[:-1]
            tots[jj, b] = _round_up(int(c[-1]), 128)
    TOT = int(tots.sum())

    # edge positions: stream-major (j, b), window-sorted, rank within
    comb = (((core * J + j) * 2 + bank) * NW + win)
    order = np.argsort(comb, kind="stable")
    rank = np.empty(E, np.int64)
    rank[order] = _occurrence_rank(comb[order])

    # global stream offset of (j, b)
    seg_off = np.zeros((J, 2), np.int64)
    acc = 0
    for jj in range(J):
        for b in range(2):
            seg_off[jj, b] = acc
            acc += tots[jj, b]

    pos = seg_off[j, bank] + base[j, bank, win] + rank   # [E] within-core pos

    gstream = np.zeros((C, TOT), np.int16)
    gstream[core, pos] = rib.astype(np.int16)

    # build schedule + codes
    # per (j, b): tiles of 128 slots; targets = windows overlapping the tile
    sched = []
    cols = []      # list of (j, b, tile, win) in emission order
    for jj in range(J):
        banks = []
        for b in range(2):
            ntiles = int(tots[jj, b]) // 128
            # window occupancy [NW]: start/end slot of each window segment
            seg_lo = base[jj, b]
            seg_hi = base[jj, b] + cap[jj, b]
            tile_targets = []
            win_bounds = {}
            for t in range(ntiles):
                lo, hi = t * 128, (t + 1) * 128
                w_lo = int(np.searchsorted(seg_hi, lo, side="right"))
                w_hi = int(np.searchsorted(seg_lo, hi, side="left"))
                targs = []
                for w in range(w_lo, min(w_hi, NW)):
                    if cap[jj, b, w] == 0:
                        continue
                    col = len(cols)
                    cols.append((jj, b, t, w))
                    ti = len(targs)
                    targs.append((w, col))
                    if w not in win_bounds:
                        win_bounds[w] = [(t, ti), (t, ti)]
                    else:
                        win_bounds[w][1] = (t, ti)
                tile_targets.append(targs)
            banks.append((b, ntiles, tile_targets,
                          {w: tuple(v) for w, v in win_bounds.items()}))
        sched.append(banks)
    ncols = _round_up(max(len(cols), 1), 4)

    # one-hot tiles, host-built: onehots[core, p, col*128 + d] = 1 iff the
    # edge at (tile, p) belongs to window w(col) at local row d
    import ml_dtypes

    onehots = np.zeros((C, 128, ncols * 128), ml_dtypes.bfloat16)
    within = pos - seg_off[j, bank]
    tile_of_edge = within // 128
    p_of_edge = within % 128
    colmap = {}
    for ci, key in enumerate(cols):
        colmap[key] = ci
    keyarr = np.array(
        [
            colmap.get((int(jj_), int(b_), int(t_), int(w_)), -1)
            for jj_, b_, t_, w_ in zip(j, bank, tile_of_edge, win)
        ],
        dtype=np.int64,
    )
    valid = keyarr >= 0
    onehots[
        core[valid], p_of_edge[valid], keyarr[valid] * 128 + dloc[valid]
    ] = 1.0
    return sched, tots, gstream, onehots, ncols


def _idx_layout(stream):
    tot = stream.shape[0]
    assert tot % 16 == 0
    base = stream.reshape(tot // 16, 16).T
    return np.ascontiguousarray(np.tile(base, (8, 1)))


def _build_module(cfg, tots, sched, ncols, apply_gb1=False, apply_gb2=False):
    import concourse.bacc as bacc
    import concourse.mybir as mybir
    from concourse import tile

    f32 = mybir.dt.float32
    bf16 = mybir.dt.bfloat16
    i16 = mybir.dt.int16
    AF = mybir.ActivationFunctionType
    ALU = mybir.AluOpType

    TOT = int(tots.sum())

    nc = bacc.Bacc(
        "TRN2",
        target_bir_lowering=False,
        debug=False,
        num_devices=cfg.cores,
        dynamic_dma_scratch_size=cfg.scratch,
        num_swdge_queues=cfg.swq,
    )

    # packed consts layout (columns in a [128, CW] f32 tensor)
    co = {}
    off = 0
    for name, w in [
        ("W1", H), ("b1", 1), ("W2", 4 * H), ("W3", H), ("b3", 1),
        ("W4", H), ("id", H), ("iota", H),
    ]:
        co[name] = off
        off += w
    if apply_gb1:
        co["g1"] = off
        off += 4 * H
        co["bt1"] = off
        off += 4 * H
    if apply_gb2:
        co["g2"] = off
        off += H
        co["bt2"] = off
        off += H
    CW = off

    # ---- I/O ----
    rcT_d = nc.dram_tensor("rcT", [H, cfg.NCp], f32, kind="ExternalInput")
    xv_d = nc.dram_tensor("xv", [cfg.NVp, H], f32, kind="ExternalInput")
    gidx_d = nc.dram_tensor("gidx", [128, TOT // 16], i16, kind="ExternalInput")
    oh_d = nc.dram_tensor("onehots", [128, ncols * 128], bf16, kind="ExternalInput")
    cst_d = nc.dram_tensor("consts", [128, CW], f32, kind="ExternalInput")
    out_d = nc.dram_tensor("out", [cfg.NVp, H], f32, kind="ExternalOutput")

    # ---- internal DRAM ----
    m_dram = nc.dram_tensor("m_scratch", [cfg.M_ROWS, H], bf16)
    partial = [
        nc.dram_tensor(f"partial{j}", [cfg.CH_ROWS, H], bf16)
        for j in range(cfg.J)
    ]
    rs_t = [nc.dram_tensor(f"rs{j}", [cfg.S, H], bf16) for j in range(cfg.J)]

    rg = [list(range(cfg.cores))]
    NB1 = cfg.T1 // 4

    with tile.TileContext(nc) as tc:
        with tc.tile_pool(name="consts", bufs=1) as cp:
            cst_s = cp.tile([128, CW], f32)
            nc.sync.dma_start(cst_s[:], cst_d[:])
            W1_s = cst_s[:, co["W1"] : co["W1"] + H]
            b1_s = cst_s[:, co["b1"] : co["b1"] + 1]
            W2_s = cst_s[:, co["W2"] : co["W2"] + 4 * H]
            W3_s = cst_s[:, co["W3"] : co["W3"] + H]
            b3_s = cst_s[:, co["b3"] : co["b3"] + 1]
            W4_s = cst_s[:, co["W4"] : co["W4"] + H]
            id_s = cst_s[:, co["id"] : co["id"] + H]

            gidx_s = cp.tile([128, TOT // 16], i16)
            nc.sync.dma_start(gidx_s[:], gidx_d[:])

            eps_s = cp.tile([128, 1], f32)
            nc.vector.memset(eps_s[:], EPS)

            def ln_chain(stp, mv_tile, n, tag):
                sd = stp.tile([128, n], f32, tag=f"{tag}_sd")
                nc.scalar.activation(
                    sd[:], mv_tile[:, :n, 1], AF.Sqrt, bias=eps_s[:]
                )
                rsig = stp.tile([128, n], f32, tag=f"{tag}_rsig")
                nc.vector.reciprocal(rsig[:], sd[:])
                nmu = stp.tile([128, n], f32, tag=f"{tag}_nmu")
                nc.vector.tensor_scalar(
                    nmu[:], mv_tile[:, :n, 0], -1.0, None, ALU.mult
                )
                nmr = stp.tile([128, n], f32, tag=f"{tag}_nmr")
                nc.vector.tensor_tensor(nmr[:], nmu[:], rsig[:], ALU.mult)
                return rsig, nmr

            # ============ Phase 1: cst_send MLP ============
            with (
                tc.tile_pool(name="m1_io", bufs=3) as iop,
                tc.tile_pool(name="m1_mid", bufs=3) as midp,
                tc.tile_pool(name="m1_stat", bufs=3) as stp,
                tc.tile_pool(name="ps1", bufs=2, space="PSUM") as psA,
                tc.tile_pool(name="ps2", bufs=5, space="PSUM") as psB,
            ):
                for blk in range(NB1):
                    xt = iop.tile([128, 512], f32, tag="xt")
                    nc.sync.dma_start(
                        xt[:], rcT_d[:, blk * 512 : (blk + 1) * 512]
                    )
                    ps_h1 = psA.tile([128, 512], f32, tag="ps_h1")
                    nc.tensor.matmul(
                        ps_h1[:], W1_s, xt[:], start=True, stop=True
                    )
                    h1 = midp.tile([128, 512], f32, tag="h1")
                    nc.scalar.activation(h1[:], ps_h1[:], AF.Relu, bias=b1_s)
                    mn4 = midp.tile([128, 2048], bf16, tag="mn4")
                    sT = stp.tile([128, 4, 6], f32, tag="sT")
                    mv = stp.tile([128, 4, 2], f32, tag="mv")
                    ps_ms = []
                    for t in range(4):
                        ps_m = psB.tile([128, 512], f32, tag="ps_m")
                        ps_ms.append(ps_m)
                        nc.tensor.matmul(
                            ps_m[:],
                            h1[:, t * 128 : (t + 1) * 128],
                            W2_s,
                            start=True,
                            stop=True,
                        )
                        nc.vector.bn_stats(sT[:, t, :], ps_m[:])
                        nc.vector.bn_aggr(mv[:, t, :], sT[:, t, :])
                    rsig, nmr = ln_chain(stp, mv, 4, "p1")
                    for t in range(4):
                        mnv = mn4[:, t * 512 : (t + 1) * 512]
                        nc.scalar.activation(
                            mnv, ps_ms[t][:], AF.Identity,
                            bias=nmr[:, t : t + 1],
                            scale=rsig[:, t : t + 1],
                        )
                        if apply_gb1:
                            g1_s = cst_s[:, co["g1"] : co["g1"] + 4 * H]
                            bt1_s = cst_s[:, co["bt1"] : co["bt1"] + 4 * H]
                            nc.vector.tensor_tensor(mnv, mnv, g1_s, ALU.mult)
                            nc.vector.tensor_tensor(mnv, mnv, bt1_s, ALU.add)
                    dstv = m_dram[blk * 2048 : (blk + 1) * 2048, :].rearrange(
                        "(t c s) h -> c t (s h)", t=4, c=128, s=4
                    )
                    nc.sync.dma_start(
                        dstv, mn4[:].rearrange("c (t f) -> c t f", t=4)
                    )

            # ==== Phase 2: dst-sorted gather + one-hot matmul reduce ====
            # ==== interleaved with Phase 3 (val_rec MLP per chunk)   ====
            OHG = 32   # one-hot tiles per SBUF load group
            with (
                tc.tile_pool(name="slots", bufs=3) as sp,
                tc.tile_pool(name="cbuf", bufs=1) as cbp,
                tc.tile_pool(name="oh", bufs=3) as ohp,
                tc.tile_pool(name="stg", bufs=3) as stgp,
                tc.tile_pool(name="m2_io", bufs=3) as iop2,
                tc.tile_pool(name="m2_mid", bufs=3) as midp2,
                tc.tile_pool(name="m2_stat", bufs=3) as stp2,
                tc.tile_pool(name="psw", bufs=cfg.psum_bufs, space="PSUM") as psW,
                tc.tile_pool(name="ps3", bufs=2, space="PSUM") as psC,
                tc.tile_pool(name="ps4", bufs=2, space="PSUM") as psD,
            ):
                chunkbuf = cbp.tile([128, cfg.NWIN, H], bf16)

                def phase3_chunk(j):
                    o = 0
                    while o < cfg.S:
                        nt = min(4, (cfg.S - o) // 128)
                        F = nt * 128
                        r0 = j * cfg.S + o
                        xt2 = iop2.tile([128, 512], f32, tag="xt2")
                        nc.sync.dma_start(
                            xt2[:, :F].rearrange("p (a h) -> p a h", a=nt),
                            xv_d[r0 : r0 + F, :].rearrange(
                                "(a p) h -> p a h", p=128
                            ),
                        )
                        rv = iop2.tile([128, 512], bf16, tag="rv")
                        nc.sync.dma_start(
                            rv[:, :F].rearrange("p (a h) -> p a h", a=nt),
                            rs_t[j][o : o + F, :].rearrange(
                                "(a p) h -> p a h", p=128
                            ),
                        )
                        hh = midp2.tile([128, 512], f32, tag="hh")
                        nc.vector.tensor_tensor(
                            hh[:, :F], xt2[:, :F], rv[:, :F], ALU.add
                        )
                        ps_hT = psC.tile([128, 512], f32, tag="ps_hT")
                        for i in range(nt):
                            nc.tensor.transpose(
                                ps_hT[:, i * 128 : (i + 1) * 128],
                                hh[:, i * 128 : (i + 1) * 128],
                                id_s,
                            )
                        hT = midp2.tile([128, 512], f32, tag="hT")
                        nc.scalar.copy(hT[:, :F], ps_hT[:, :F])
                        ps_h2 = psD.tile([128, 512], f32, tag="ps_h2")
                        nc.tensor.matmul(
                            ps_h2[:, :F], W3_s, hT[:, :F], start=True, stop=True
                        )
                        h2 = midp2.tile([128, 512], f32, tag="h2")
                        nc.scalar.activation(
                            h2[:, :F], ps_h2[:, :F], AF.Relu, bias=b3_s
                        )
                        ps_o = psC.tile([128, 512], f32, tag="ps_hT")
                        sT2 = stp2.tile([128, 4, 6], f32, tag="sT2")
                        mv2 = stp2.tile([128, 4, 2], f32, tag="mv2")
                        for i in range(nt):
                            nc.tensor.matmul(
                                ps_o[:, i * 128 : (i + 1) * 128],
                                h2[:, i * 128 : (i + 1) * 128],
                                W4_s,
                                start=True,
                                stop=True,
                            )
                            po = ps_o[:, i * 128 : (i + 1) * 128]
                            nc.vector.bn_stats(sT2[:, i, :], po)
                            nc.vector.bn_aggr(mv2[:, i, :], sT2[:, i, :])
                        rsig2, nmr2 = ln_chain(stp2, mv2, nt, "p3")
                        on = midp2.tile([128, 512], f32, tag="on")
                        for i in range(nt):
                            onv = on[:, i * 128 : (i + 1) * 128]
                            nc.scalar.activation(
                                onv,
                                ps_o[:, i * 128 : (i + 1) * 128],
                                AF.Identity,
                                bias=nmr2[:, i : i + 1],
                                scale=rsig2[:, i : i + 1],
                            )
                            if apply_gb2:
                                g2_s = cst_s[:, co["g2"] : co["g2"] + H]
                                bt2_s = cst_s[:, co["bt2"] : co["bt2"] + H]
                                nc.vector.tensor_tensor(onv, onv, g2_s, ALU.mult)
                                nc.vector.tensor_tensor(onv, onv, bt2_s, ALU.add)
                        nc.vector.tensor_tensor(
                            on[:, :F], on[:, :F], xt2[:, :F], ALU.add
                        )
                        nc.scalar.dma_start(
                            out_d[r0 : r0 + F, :].rearrange(
                                "(a p) h -> p a h", p=128
                            ),
                            on[:, :F].rearrange("p (a h) -> p a h", a=nt),
                        )
                        o += F

                seg_start = 0
                col_base = 0   # running one-hot column index
                for j in range(cfg.J):
                    # fresh accumulation chunk
                    nc.vector.memset(
                        chunkbuf[:].rearrange("p a h -> p (a h)"), 0.0
                    )
                    # psum window tiles live across tiles of one (j,b) pass
                    win_psum = {}
                    for (b, ntiles, tile_targets, win_bounds) in sched[j]:
                        nslots = ntiles * 128
                        # gather in sub-sized calls; slot tile ring
                        st_tiles = []
                        for w0 in range(0, nslots, cfg.sub):
                            wl = min(cfg.sub, nslots - w0)
                            st = sp.tile(
                                [128, cfg.sub // 128, 128], bf16, tag="slots"
                            )
                            gq = (w0 // cfg.sub) % cfg.swq
                            nc.gpsimd.dma_gather(
                                st[:, : wl // 128, :],
                                m_dram[
                                    b * cfg.BANK_ROWS : (b + 1) * cfg.BANK_ROWS,
                                    :,
                                ],
                                gidx_s[
                                    :,
                                    (seg_start + w0) // 16 : (seg_start + w0 + wl)
                                    // 16,
                                ],
                                wl,
                                wl,
                                H,
                                queue_num=gq,
                            )
                            st_tiles.append((w0, st))

                        # number of one-hot targets in this (j, b)
                        njb = sum(len(tt) for tt in tile_targets)
                        oh_tiles = {}
                        stage = None
                        stage_w0 = None
                        npend = 0

                        def flush_stage():
                            nonlocal stage, stage_w0, npend
                            if npend:
                                v = chunkbuf[:, stage_w0 : stage_w0 + npend, :]
                                nc.vector.tensor_tensor(
                                    v.rearrange("p a h -> p (a h)"),
                                    v.rearrange("p a h -> p (a h)"),
                                    stage[:, :npend, :].rearrange(
                                        "p a h -> p (a h)"
                                    ),
                                    ALU.add,
                                )
                            stage = None
                            stage_w0 = None
                            npend = 0

                        ci_emitted = 0
                        for t in range(ntiles):
                            ci = t * 128 // cfg.sub
                            w0, st = st_tiles[ci]
                            msg = st[:, (t * 128 - w0) // 128, :]
                            for ti, (w, col) in enumerate(tile_targets[t]):
                                first = win_bounds[w][0] == (t, ti)
                                last = win_bounds[w][1] == (t, ti)
                                gi = (col - col_base) // OHG
                                if gi not in oh_tiles:
                                    g0 = col_base + gi * OHG
                                    gn = min(OHG, col_base + njb - g0)
                                    oht = ohp.tile(
                                        [128, OHG * 128], bf16, tag="oht"
                                    )
                                    nc.sync.dma_start(
                                        oht[:, : gn * 128],
                                        oh_d[:, g0 * 128 : (g0 + gn) * 128],
                                    )
                                    oh_tiles[gi] = oht
                                oht = oh_tiles[gi]
                                oslice = ((col - col_base) % OHG) * 128
                                if first:
                                    pw = psW.tile([128, 128], f32, tag="pw")
                                    win_psum[w] = pw
                                pw = win_psum[w]
                                nc.tensor.matmul(
                                    pw[:],
                                    oht[:, oslice : oslice + 128],
                                    msg,
                                    start=first,
                                    stop=last,
                                )
                                if last:
                                    if stage is None or w != stage_w0 + npend or npend == 8:
                                        flush_stage()
                                        stage = stgp.tile(
                                            [128, 8, 128], f32, tag="stg"
                                        )
                                        stage_w0 = w
                                    nc.scalar.copy(
                                        stage[:, w - stage_w0, :], pw[:]
                                    )
                                    npend += 1
                                    del win_psum[w]
                        flush_stage()
                        col_base += njb
                        seg_start += nslots

                    # chunk done: write partial[j], RS, phase 3
                    nc.scalar.dma_start(
                        partial[j][:, :].rearrange(
                            "(a p) h -> p a h", p=128
                        ),
                        chunkbuf[:]
                        .rearrange("p a h -> p a h"),
                    )
                    nc.gpsimd.collective_compute(
                        "ReduceScatter",
                        mybir.AluOpType.add,
                        replica_groups=rg,
                        ins=[partial[j][:, :]],
                        outs=[rs_t[j][:, :]],
                    )
                    phase3_chunk(j)

    nc.compile()
    return nc


def _prep_inputs(cfg, inputs):
    x_val = np.ascontiguousarray(np.asarray(inputs["x_val"], np.float32))
    r_cst = np.ascontiguousarray(np.asarray(inputs["r_cst"], np.float32))
    edges = np.asarray(inputs["cst_edges"]).astype(np.int64)
    le = np.asarray(inputs["LE"]).astype(np.int64)
    pe = np.asarray(inputs["PE"]).astype(np.int64)
    slot = 2 * le + pe

    sched, tots, gstream, onehots, ncols = _build_plan(
        cfg, edges[0], edges[1], slot
    )

    W1 = np.asarray(inputs["W1"], np.float32)
    b1 = np.asarray(inputs["b1"], np.float32).reshape(H, 1)
    W2 = np.asarray(inputs["W2"], np.float32)
    W3 = np.asarray(inputs["W3"], np.float32)
    b3 = np.asarray(inputs["b3"], np.float32).reshape(H, 1)
    W4 = np.asarray(inputs["W4"], np.float32)
    g1 = np.asarray(inputs["g1"], np.float32)
    bt1 = np.asarray(inputs["bt1"], np.float32)
    g2 = np.asarray(inputs["g2"], np.float32)
    bt2 = np.asarray(inputs["bt2"], np.float32)
    apply_gb1 = not (np.all(g1 == 1.0) and np.all(bt1 == 0.0))
    apply_gb2 = not (np.all(g2 == 1.0) and np.all(bt2 == 0.0))

    parts = [
        W1, b1, W2, W3, b3, W4,
        np.eye(128, dtype=np.float32),
        np.broadcast_to(np.arange(128, dtype=np.float32), (128, 128)),
    ]
    if apply_gb1:
        parts += [
            np.broadcast_to(g1, (128, 4 * H)),
            np.broadcast_to(bt1, (128, 4 * H)),
        ]
    if apply_gb2:
        parts += [
            np.broadcast_to(g2, (128, H)),
            np.broadcast_to(bt2, (128, H)),
        ]
    consts = np.ascontiguousarray(
        np.concatenate(parts, axis=1, dtype=np.float32)
    )

    import ml_dtypes

    in_maps = []
    for c in range(cfg.cores):
        rc = r_cst[c * cfg.NCs : (c + 1) * cfg.NCs]
        rcT = np.zeros((H, cfg.NCp), np.float32)
        rcT[:, : cfg.NCs] = rc.T
        xv = np.zeros((cfg.NVp, H), np.float32)
        xv[: cfg.NVs] = x_val[c * cfg.NVs : (c + 1) * cfg.NVs]
        in_maps.append(
            {
                "rcT": np.ascontiguousarray(rcT),
                "xv": xv,
                "gidx": _idx_layout(gstream[c]),
                "onehots": np.ascontiguousarray(onehots[c]),
                "consts": consts,
            }
        )
    return in_maps, tots, sched, ncols, apply_gb1, apply_gb2


def run(inputs, cfg=None, trace=False):
    from concourse.bass_utils import run_bass_kernel_spmd

    cfg = cfg or Cfg()
    in_maps, tots, sched, ncols, gb1, gb2 = _prep_inputs(cfg, inputs)
    nc = _build_module(cfg, tots, sched, ncols, gb1, gb2)
    res = run_bass_kernel_spmd(
        nc, in_maps, core_ids=list(range(cfg.cores)), trace=trace
    )
    out = np.concatenate(
        [res.results[c]["out"][: cfg.NVs] for c in range(cfg.cores)], axis=0
    )
    return out, res


def kernel(**inputs) -> np.ndarray:
    out, _ = run(inputs)
    return out
